# revision 6
# baseline (speedup 1.0000x reference)
"""Trainium2 Bass kernel for nn_ConstrainedEnhancementModel (v4).

Device computes only the expensive part: the bf16 encoder MLP
(8192->1024->512->256->512->1024, K-sharded L1 + AllReduce) and the
fp8-DoubleRow 1024x131072 output projection (column-sharded, core c owns
output timesteps [c*512, (c+1)*512)), returning RAW decoded = d2 @ w6
(pre-bias) as fp8 in transposed [cols, batch] layout.

Host side then adds b6 in fp32 and applies the elementwise constraint
blend (knot passthrough / 0.8*interp + 0.2*decoded / tail) exactly --
this is O(B*H*F) numpy. Rationale: the blend folded on-device (v2)
forced a bf16 output stream (16.8MB/core) plus interp matmuls+tables;
returning fp8 decoded halves the output stream to 8.4MB/core and drops
27us of tensor-engine time, and the measured rel err stays ~8e-3 since
decoded carries only 0.2 weight in the blend.

w6 is prescaled by SW=512 (clears the fp8e4 subnormal floor at
std(w6)=1/32); the psum->sbuf copy undoes it.
"""

from contextlib import ExitStack

import numpy as np
import ml_dtypes

import concourse.bacc as bacc
import concourse.mybir as mybir
import concourse.tile as tile
from concourse.bass import ds, ts
from concourse.bass_utils import run_bass_kernel_spmd

DT = mybir.dt

B, L, F, H, HID = 512, 256, 32, 4096, 512
UP = H // L          # 16 timesteps per low-res segment
LF = L * F           # 8192
HF = H * F           # 131072
NCORES = 8
COLS = HF // NCORES  # 16384 output rows per core (outT orientation)
SEGC = UP * F        # 512 output rows per segment
NSEG = COLS // SEGC  # 32 segments per core
GRP = 4              # segments per w6 DMA (2MB transfers)
NBIAS = 8 + 4 + 2 + 4 + 8  # packed bias columns
SW = 512.0           # w6 prescale (fp8 subnormal floor), undone on copy-out

_CACHE: dict = {}


def _build_program(reps=1, phase="all"):
    """One SPMD program; per-core data differences live in the inputs.

    reps>1 repeats the whole body back-to-back inside one NEFF (timing).
    phase: "all" | "enc" (encoder only) | "big" (projection only, dummy d2).
    """
    bf16, f32, f8 = DT.bfloat16, DT.float32, DT.float8e4
    nc = bacc.Bacc("TRN2", target_bir_lowering=False, debug=False, num_devices=NCORES)

    KSH = LF // NCORES  # 1024 contraction rows of layer 1 per core
    xTs = nc.dram_tensor("xTs", [KSH, B], bf16, kind="ExternalInput")
    w1s = nc.dram_tensor("w1s", [KSH, 2 * HID], bf16, kind="ExternalInput")
    arin = nc.dram_tensor("arin", [2 * HID, B], bf16)
    arout = nc.dram_tensor("arout", [2 * HID, B], bf16, addr_space="Shared")
    w2 = nc.dram_tensor("w2", [2 * HID, HID], bf16, kind="ExternalInput")
    w3 = nc.dram_tensor("w3", [HID, HID // 2], bf16, kind="ExternalInput")
    w4 = nc.dram_tensor("w4", [HID // 2, HID], bf16, kind="ExternalInput")
    w5 = nc.dram_tensor("w5", [HID, 2 * HID], bf16, kind="ExternalInput")
    bpk = nc.dram_tensor("bpk", [128, NBIAS], f32, kind="ExternalInput")
    w6dr = nc.dram_tensor(
        "w6dr", [NSEG // GRP, 128, GRP, 4, 4, 2, 128], f8, kind="ExternalInput"
    )
    out = nc.dram_tensor("out", [COLS, B], f8, kind="ExternalOutput")

    RELU = mybir.ActivationFunctionType.Relu
    IDENT = mybir.ActivationFunctionType.Identity
    DR = mybir.MatmulPerfMode.DoubleRow

    with tile.TileContext(nc) as tc:

        def _one_rep(rep, ctx):
            psum = ctx.enter_context(
                tc.tile_pool(name=f"psum{rep}", bufs=8, space="PSUM")
            )
            xpool = ctx.enter_context(tc.tile_pool(name=f"xpool{rep}", bufs=4))
            wpool = ctx.enter_context(tc.tile_pool(name=f"wpool{rep}", bufs=5))
            scratch = ctx.enter_context(tc.tile_pool(name=f"scratch{rep}", bufs=2))
            acts = ctx.enter_context(tc.tile_pool(name=f"acts{rep}", bufs=1))
            bpool = ctx.enter_context(tc.tile_pool(name=f"bpool{rep}", bufs=1))
            w6pool = ctx.enter_context(tc.tile_pool(name=f"w6pool{rep}", bufs=4))
            opool = ctx.enter_context(tc.tile_pool(name=f"opool{rep}", bufs=4))

            btile = bpool.tile([128, NBIAS], f32, name="btile")
            nc.scalar.dma_start(btile[:], bpk[:])
            boff = {1: 0, 2: 8, 3: 12, 4: 14, 5: 18}

            def _enc():
                # ---- L1: K-sharded partial matmul + AllReduce over 8 cores ----
                ps1 = [
                    psum.tile([128, B], f32, tag="psum", name=f"ps1_{m}")
                    for m in range(8)
                ]
                for kc in range(4):
                    e1 = nc.sync if kc % 2 == 0 else nc.scalar
                    e2 = nc.scalar if kc % 2 == 0 else nc.sync
                    xt = xpool.tile([128, 2, B], bf16, name=f"xt{kc}", tag="xt")
                    e2.dma_start(
                        xt[:],
                        xTs[ds(kc * 256, 256), :].rearrange("(k p) d -> p k d", p=128),
                    )
                    w1t = wpool.tile([128, 2, 2 * HID], bf16, name=f"w1t{kc}", tag="w")
                    e1.dma_start(
                        w1t[:],
                        w1s[ds(kc * 256, 256), :].rearrange("(k p) d -> p k d", p=128),
                    )
                    for k4 in range(2):
                        for m in range(8):
                            nc.tensor.matmul(
                                ps1[m][:],
                                w1t[:, k4, ts(m, 128)],
                                xt[:, k4, :],
                                start=(kc == 0 and k4 == 0),
                                stop=(kc == 3 and k4 == 1),
                            )
                hp = scratch.tile([128, 8, B], bf16, tag="s", name="hp")
                for m in range(8):
                    nc.vector.tensor_copy(hp[:, m, :], ps1[m][:])
                nc.sync.dma_start(
                    arin[ds(0, 512), :].rearrange("(m p) d -> p m d", p=128),
                    hp[:, 0:4, :],
                )
                nc.scalar.dma_start(
                    arin[ds(512, 512), :].rearrange("(m p) d -> p m d", p=128),
                    hp[:, 4:8, :],
                )
                nc.gpsimd.collective_compute(
                    "AllReduce",
                    mybir.AluOpType.add,
                    replica_groups=[list(range(NCORES))],
                    ins=[arin[:]],
                    outs=[arout[:]],
                )
                htmp = scratch.tile([128, 8, B], bf16, tag="s", name="htmp")
                nc.sync.dma_start(
                    htmp[:, 0:4, :],
                    arout[ds(0, 512), :].rearrange("(m p) d -> p m d", p=128),
                )
                nc.scalar.dma_start(
                    htmp[:, 4:8, :],
                    arout[ds(512, 512), :].rearrange("(m p) d -> p m d", p=128),
                )
                h1 = scratch.tile([128, 8, B], bf16, tag="s", name="h1")
                for m in range(8):
                    nc.scalar.activation(
                        h1[:, m, :], htmp[:, m, :], RELU, bias=btile[:, m : m + 1]
                    )

                # ---- L2..L5 (one DMA per layer, weights via shared pool) ----
                def mlp_layer(w_dram, k_tiles, m_tiles, rhs, b_idx, func, name, pool,
                              out_dtype=bf16):
                    o = pool.tile(
                        [128, m_tiles, B], out_dtype,
                        tag="s" if pool is scratch else name, name=name,
                    )
                    ps = [
                        psum.tile([128, B], f32, tag="psum", name=f"ps_{name}_{m}")
                        for m in range(m_tiles)
                    ]
                    for kc in range(0, k_tiles, 2):
                        kw = min(2, k_tiles - kc)
                        wt = wpool.tile(
                            [128, kw, m_tiles * 128], bf16, tag="w",
                            name=f"w_{name}_{kc}",
                        )
                        eng = nc.sync if (kc // 2) % 2 == 0 else nc.scalar
                        eng.dma_start(
                            wt[:],
                            w_dram[ds(kc * 128, kw * 128), :].rearrange(
                                "(k p) d -> p k d", p=128
                            ),
                        )
                        for ki in range(kw):
                            for m in range(m_tiles):
                                nc.tensor.matmul(
                                    ps[m][:],
                                    wt[:, ki, ts(m, 128)],
                                    rhs[:, kc + ki, :],
                                    start=(kc + ki == 0),
                                    stop=(kc + ki == k_tiles - 1),
                                )
                    ob = boff[b_idx]
                    for m in range(m_tiles):
                        nc.scalar.activation(
                            o[:, m, :], ps[m][:], func,
                            bias=btile[:, ob + m : ob + m + 1],
                        )
                    return o

                h2 = mlp_layer(w2, 8, 4, h1, 2, RELU, "h2", scratch)
                ft = mlp_layer(w3, 4, 2, h2, 3, IDENT, "ft", scratch)
                d1 = mlp_layer(w4, 2, 4, ft, 4, RELU, "d1", scratch)
                return mlp_layer(w5, 4, 8, d1, 5, RELU, "d2", acts, out_dtype=f8)

            def _big(d2):
                # ---- big projection (fp8 DoubleRow), raw decoded out ----
                inv = 1.0 / SW
                for g in range(NSEG // GRP):
                    w6t = w6pool.tile(
                        [128, GRP, 4, 4, 2, 128], f8, name=f"w6t{g}", tag="w6"
                    )
                    nc.sync.dma_start(w6t[:], w6dr[g])
                    for sl in range(GRP):
                        s = g * GRP + sl
                        obuf = opool.tile([128, 4, SEGC], f8, tag="ot", name=f"ot{s}")
                        for jt in range(4):
                            ps = psum.tile(
                                [128, SEGC], f32, tag="psum", name=f"pso_{s}_{jt}"
                            )
                            for kt in range(4):
                                nc.tensor.matmul(
                                    ps[:],
                                    w6t[:, sl, jt, kt, :, :],
                                    d2[:, ds(2 * kt, 2), :],
                                    perf_mode=DR,
                                    start=(kt == 0),
                                    stop=(kt == 3),
                                )
                            if jt % 2 == 0:
                                nc.scalar.activation(
                                    obuf[:, jt, :], ps[:], IDENT, scale=inv
                                )
                            else:
                                nc.vector.tensor_scalar_mul(obuf[:, jt, :], ps[:], inv)
                        nc.scalar.dma_start(
                            out[ds(s * SEGC, SEGC), :].rearrange(
                                "(a p) d -> p a d", p=128
                            ),
                            obuf[:],
                        )

            if phase == "enc":
                d2 = _enc()
                otx = opool.tile([128, B], f8, name="otx", tag="ot")
                nc.vector.tensor_copy(otx[:], d2[:, 0, :])
                nc.sync.dma_start(out[ts(0, 128), :], otx[:])
            elif phase == "big":
                d2 = acts.tile([128, 8, B], f8, tag="d2", name="d2")
                nc.vector.memset(d2[:], 0.5)
                _big(d2)
            else:
                _big(_enc())

        for _rep in range(reps):
            with ExitStack() as _ctx:
                _one_rep(_rep, _ctx)

    nc.compile()
    return nc


def _host_prep(inputs):
    """Shard + quantize. Returns per-core input maps."""
    BF = ml_dtypes.bfloat16
    F8 = ml_dtypes.float8_e4m3

    x = np.ascontiguousarray(inputs["low_res_data"], dtype=np.float32)
    x2d = x.reshape(B, LF)
    xTa = np.ascontiguousarray(x2d.T)  # (8192, 512)
    w6 = np.asarray(inputs["w6"], dtype=np.float32)

    bpk = np.zeros((128, NBIAS), np.float32)
    off = 0
    for i in (1, 2, 3, 4, 5):
        bv = np.asarray(inputs[f"b{i}"], np.float32)
        m = bv.shape[0] // 128
        bpk[:, off : off + m] = bv.reshape(m, 128).T
        off += m

    w1f = np.asarray(inputs["w1"], np.float32)
    shared = {
        "w2": np.asarray(inputs["w2"], np.float32).astype(BF),
        "w3": np.asarray(inputs["w3"], np.float32).astype(BF),
        "w4": np.asarray(inputs["w4"], np.float32).astype(BF),
        "w5": np.asarray(inputs["w5"], np.float32).astype(BF),
        "bpk": bpk,
    }

    in_maps = []
    for c in range(NCORES):
        j0 = c * COLS
        # DoubleRow pack: value at [g,p,sl,jt,kt,i,j] =
        #   w6[256kt+128i+p, j0 + 512*(4g+sl) + 128jt + j] * SW
        w6c = np.ascontiguousarray(w6[:, j0 : j0 + COLS]) * SW
        arr = w6c.reshape(4, 2, 128, NSEG, 4, 128)  # [kt, i, p, s, jt, j]
        arr = arr.transpose(3, 2, 4, 0, 1, 5)  # [s, p, jt, kt, i, j]
        arr = arr.reshape(NSEG // GRP, GRP, 128, 4 * 4 * 2 * 128).transpose(0, 2, 1, 3)
        w6drc = np.ascontiguousarray(arr).astype(F8).reshape(
            NSEG // GRP, 128, GRP, 4, 4, 2, 128
        )

        xTsl = np.ascontiguousarray(
            xTa[c * (LF // NCORES) : (c + 1) * (LF // NCORES)]
        ).astype(BF)
        w1sl = np.ascontiguousarray(
            w1f[c * (LF // NCORES) : (c + 1) * (LF // NCORES)]
        ).astype(BF)

        in_maps.append({**shared, "w6dr": w6drc, "xTs": xTsl, "w1s": w1sl})
    return in_maps


def kernel(**inputs):
    if "nc" not in _CACHE:
        _CACHE["nc"] = _build_program()
    nc = _CACHE["nc"]
    in_maps = _host_prep(inputs)
    res = run_bass_kernel_spmd(nc, in_maps, list(range(NCORES)))

    # gather raw decoded (outT fp8), add b6, apply constraint blend on host
    b6 = np.asarray(inputs["b6"], np.float32)
    decoded = np.empty((B, H, F), np.float32)
    HS = H // NCORES
    for c in range(NCORES):
        oc = np.asarray(res.results[c]["out"]).astype(np.float32)  # (COLS, B)
        decoded[:, c * HS : (c + 1) * HS, :] = oc.T.reshape(B, HS, F)
    decoded += b6.reshape(1, H, F)

    x = np.asarray(inputs["low_res_data"], np.float32)
    t = np.arange(H)
    seg = t // UP
    is_knot = (t % UP) == 0
    last_knot = (L - 1) * UP
    in_segment = (~is_knot) & (t < last_knot)
    seg_lo = np.minimum(seg, L - 2)
    alpha = ((t % UP).astype(np.float32) / UP)[None, :, None]
    linear_interp = (1.0 - alpha) * x[:, seg_lo, :] + alpha * x[:, seg_lo + 1, :]
    smoothed = 0.8 * linear_interp + 0.2 * decoded
    out = np.where(
        is_knot[None, :, None],
        x[:, seg, :],
        np.where(in_segment[None, :, None], smoothed, decoded),
    )
    return np.ascontiguousarray(out, np.float32)
